# revision 26
# baseline (speedup 1.0000x reference)
"""Bass/Trainium2 LSTM encoder kernel.

Problem: nn_Encoder (LSTM): input [B=4096, T=512, IN=22], hidden H=64,
torch gate order i,f,g,o. Output: hidden states [B, T, H].

Sharding: data-parallel over batch across 8 NeuronCores (512 batch rows per
core, split into two software-pipelined streams of 256). Weights replicated.
The T=512 recurrence runs sequentially per core.

Per-core structure (feature-on-partition, batch in the free dim):
  - x host-transposed to xT [T, 23, B]; row 22 is ones, so the bias rides the
    x-matmul (K=23). All matmul operands are float32r (TF32-class, 4x the
    fp32 PE rate at N>=256; ~1e-4 relative rounding).
  - Stationary S1/S2 [128, 128]: rows 0:22 W_ihT gate-chunk, row 22 combined
    bias, rows 64:128 W_hhT gate-chunk. Per stream-step, two row-tiled
    matmuls per chunk (x-part at array rows 0:23, h-part at rows 64:128)
    accumulate one psum bank [128, 512] (chunk1 cols 0:256, chunk2 256:512).
  - Uniform-tanh trick: sigmoid-gate rows (i, f, o) of W and bias are
    pre-scaled 0.5 on the host (sigmoid(z) = 0.5 tanh(0.5 z) + 0.5), so ONE
    tanh activation over the whole psum produces all gates: G = [ti;tf|tg;to].
  - Scaled state C = 2c and history h' = 2h (host halves the output):
      u' = (ti+1)*tg           scalar_tensor_tensor on DVE
      W  = (tf+1)*C            scalar_tensor_tensor on DVE
      C  = 0.5 W + u'          matmul against a constant [0.5 I; I] matrix
                               (TensorE is idle; removes a DVE chain stage);
                               C lives in PSUM
      tc = tanh(0.5 C)         activation with input scale, PSUM-sourced
      h' = (to+1)*tc           scalar_tensor_tensor on DVE
  - Two batch streams of 256 are interleaved so the per-step dependency
    chain of one stream overlaps the other stream's engine work.
  - h' written into an SBUF history strip (base partition 64, aligned with
    the W_hh array rows), DMA'd out every TC steps as hs [T, H, B]; the host
    transposes back to [B, T, H] and multiplies by 0.5.
Walrus in this container accepts at most ONE semaphore wait per instruction;
_split_waits post-processes Tile's output to satisfy that.
"""

import numpy as np

import bass_rust
import concourse.bass as bass
import concourse.mybir as mybir
import concourse.tile as tile
import concourse.bass_utils as bass_utils

N_CORES = 8
B_FULL, T, IN, H = 4096, 512, 22, 64
B = B_FULL // N_CORES          # batch per core
BS = B // 2                    # batch per stream
KX = IN + 1                    # x rows + ones row
TC = 16                        # timesteps per DMA chunk
F32 = mybir.dt.float32

_cache = {}


def _split_waits(nc, max_waits=1):
    """walrus here allows one sem-wait per instruction; split extras into
    preceding same-engine NOPs."""
    for f in nc.m.functions:
        for bb in f.blocks:
            insts = bb.instructions
            changed = False
            out = []
            for inst in insts:
                si = inst.sync_info
                if si is not None and si.on_wait and len(si.on_wait) > max_waits:
                    waits = list(si.on_wait)
                    head, rest = waits[:-max_waits], waits[-max_waits:]
                    for i in range(0, len(head), max_waits):
                        nop = mybir.InstNoOp(name=nc.get_next_instruction_name())
                        nop.engine = inst.engine
                        nop.sync_info = bass_rust.SyncInfo(
                            on_wait=head[i:i + max_waits], on_update=[])
                        out.append(nop)
                    inst.sync_info = bass_rust.SyncInfo(
                        on_wait=rest, on_update=list(si.on_update))
                    changed = True
                out.append(inst)
            if changed:
                cur = bb.instructions
                del cur[:]
                cur.extend(out)


def _build():
    if "nc" in _cache:
        return _cache["nc"]

    nc = bass.Bass("TRN2", target_bir_lowering=False, debug=False,
                   enable_asserts=False, num_devices=1)

    xT_d = nc.dram_tensor("xT", [T, KX, B], F32, kind="ExternalInput").ap()
    s1_d = nc.dram_tensor("S1", [128, 128], F32, kind="ExternalInput").ap()
    s2_d = nc.dram_tensor("S2", [128, 128], F32, kind="ExternalInput").ap()
    p_d = nc.dram_tensor("P", [128, 128], F32, kind="ExternalInput").ap()
    hs_d = nc.dram_tensor("hs", [T, H, B], F32, kind="ExternalOutput").ap()

    TANH = mybir.ActivationFunctionType.Tanh
    F32R = mybir.dt.float32r
    BF16 = mybir.dt.bfloat16
    ADD = mybir.AluOpType.add
    MUL = mybir.AluOpType.mult

    n_chunks = T // TC

    with tile.TileContext(nc) as tc:
        with (
            tc.tile_pool(name="const", bufs=1) as cpool,
            tc.tile_pool(name="xin", bufs=3) as xpool,
            tc.tile_pool(name="hh", bufs=2) as hpool,
            tc.tile_pool(name="gates", bufs=6) as gpool,
            tc.tile_pool(name="tmp", bufs=8) as tpool,
            tc.tile_pool(name="ps", bufs=4, space="PSUM") as pspool,
        ):
            s1 = cpool.tile([128, 128], F32R, tag="s1")
            s2 = cpool.tile([128, 128], F32R, tag="s2")
            pmat = cpool.tile([128, 128], F32R, tag="pmat")
            nc.sync.dma_start(s1[:], s1_d[:].bitcast(F32R))
            nc.sync.dma_start(s2[:], s2_d[:].bitcast(F32R))
            nc.sync.dma_start(pmat[:], p_d[:].bitcast(F32R))

            # c state lives in PSUM, written by a PE-add matmul; c_prev[s]
            # is the AP of the previous step's psum c tile (None -> zeros)
            c_prev = [None, None]

            h_prev = [None, None]   # AP of h_{t-1} per stream
            for ci in range(n_chunks):
                xch = xpool.tile([KX, TC * B], F32R, tag="x")
                nc.sync.dma_start(
                    xch[:].rearrange("k (t b) -> k t b", t=TC),
                    xT_d[ci * TC:(ci + 1) * TC].rearrange("t k b -> k t b")
                    .bitcast(F32R),
                )
                hh = hpool.tile([128, TC * B], F32, tag="h")
                for j in range(TC):
                    for s in (0, 1):
                        off = j * B + s * BS
                        rx = xch[:, off:off + BS]
                        ps = pspool.tile([128, 2 * BS], F32, tag="ps")
                        first = h_prev[s] is None
                        nc.tensor.matmul(ps[:, 0:BS], s1[0:KX, :], rx,
                                         start=True, stop=first,
                                         tile_position=(0, 0))
                        if not first:
                            nc.tensor.matmul(ps[:, 0:BS], s1[64:128, :],
                                             h_prev[s], start=False, stop=True,
                                             tile_position=(64, 0))
                        nc.tensor.matmul(ps[:, BS:2 * BS], s2[0:KX, :], rx,
                                         start=True, stop=first,
                                         tile_position=(0, 0))
                        if not first:
                            nc.tensor.matmul(ps[:, BS:2 * BS], s2[64:128, :],
                                             h_prev[s], start=False, stop=True,
                                             tile_position=(64, 0))

                        # one uniform tanh over both gate chunks
                        g = gpool.tile([128, 2 * BS], F32, tag="g")
                        nc.scalar.activation(g[:], ps[:], TANH)
                        # G layout: cols 0:BS = [ti; tf], cols BS:2BS = [tg; to]
                        # State C = 2c; history h' = 2h (host halves output).
                        # u' = (ti+1)*tg = 2ig ; W = (tf+1)*C = 4fc ;
                        # C_new = 0.5*W + u' ; tc = tanh(0.5*C) ;
                        # h' = (to+1)*tc = 2h
                        # wu = [W ; u'] stacked; PE computes C = 0.5W + u'
                        wu = tpool.tile([128, BS], F32R, tag="wu")
                        nc.vector.scalar_tensor_tensor(
                            wu[64:128, :], g[0:H, 0:BS], 1.0,
                            g[0:H, BS:2 * BS], op0=ADD, op1=MUL)
                        if c_prev[s] is None:
                            nc.vector.memset(wu[0:H, :].bitcast(F32), 0.0)
                        else:
                            nc.vector.scalar_tensor_tensor(
                                wu[0:H, :], g[H:128, 0:BS], 1.0, c_prev[s],
                                op0=ADD, op1=MUL)
                        cps = pspool.tile([128, BS], F32, tag="cps")
                        nc.tensor.matmul(cps[:], pmat[:], wu[:],
                                         start=True, stop=True,
                                         tile_position=(0, 0))
                        c_prev[s] = cps[64:128, :]
                        tcb = tpool.tile([128, BS], F32, tag="tc")
                        nc.scalar.activation(tcb[64:128, :], c_prev[s], TANH,
                                             scale=0.5)
                        h_out = hh[64:128, off:off + BS].bitcast(F32R)
                        nc.vector.scalar_tensor_tensor(
                            h_out, g[H:128, BS:2 * BS], 1.0, tcb[64:128, :],
                            op0=ADD, op1=MUL)
                        h_prev[s] = h_out
                nc.sync.dma_start(
                    hs_d[ci * TC:(ci + 1) * TC].rearrange("t h b -> h t b"),
                    hh[64:128, :].rearrange("h (t b) -> h t b", t=TC),
                )

    _split_waits(nc, max_waits=1)
    _cache["nc"] = nc
    return nc


def _prep_core_inputs(input_data, W_ih, W_hh, b_ih, b_hh):
    bias = (b_ih + b_hh).astype(np.float32)           # [256]
    W_ihT = W_ih.astype(np.float32).T.copy()          # [22, 256]
    W_hhT = W_hh.astype(np.float32).T.copy()          # [64, 256]
    # scale sigmoid-gate rows (i: 0:64, f: 64:128, o: 192:256) by 0.5 for
    # the uniform-tanh trick; g rows (128:192) stay unscaled
    scale = np.ones(256, np.float32) * 0.5
    scale[128:192] = 1.0
    W_ihT *= scale
    bias *= scale
    # W_hh consumes h' = 2h from the history strip -> extra 0.5
    W_hhT *= scale * 0.5

    def stationary(lo, hi):
        s = np.zeros((128, 128), np.float32)
        s[0:IN, :] = W_ihT[:, lo:hi]
        s[IN, :] = bias[lo:hi]
        s[64:128, :] = W_hhT[:, lo:hi]
        return s

    s1 = stationary(0, 128)
    s2 = stationary(128, 256)
    # c_psum[64+m] = 0.5*wu[m] + wu[64+m]  (wu rows 0:64 = W, 64:128 = u')
    pm = np.zeros((128, 128), np.float32)
    for m in range(64):
        pm[m, 64 + m] = 0.5
        pm[64 + m, 64 + m] = 1.0

    x8 = input_data.reshape(N_CORES, B, T, IN)
    in_maps = []
    for c in range(N_CORES):
        xT = np.empty((T, KX, B), np.float32)
        xT[:, 0:IN, :] = x8[c].transpose(1, 2, 0)
        xT[:, IN, :] = 1.0
        in_maps.append({"xT": np.ascontiguousarray(xT), "S1": s1, "S2": s2,
                        "P": pm})
    return in_maps


def kernel(input_data, W_ih, W_hh, b_ih, b_hh):
    input_data = np.asarray(input_data, np.float32)
    W_ih = np.asarray(W_ih, np.float32)
    W_hh = np.asarray(W_hh, np.float32)
    b_ih = np.asarray(b_ih, np.float32)
    b_hh = np.asarray(b_hh, np.float32)

    nc = _build()
    in_maps = _prep_core_inputs(input_data, W_ih, W_hh, b_ih, b_hh)
    res = bass_utils.run_bass_kernel_spmd(nc, in_maps, core_ids=list(range(N_CORES)))
    _cache["last_results"] = res

    out = np.empty((B_FULL, T, H), np.float32)
    for c in range(N_CORES):
        hs = res.results[c]["hs"]                     # [T, H, B] (holds 2h)
        out[c * B:(c + 1) * B] = hs.transpose(2, 0, 1)
    out *= 0.5
    return out
